# revision 37
# baseline (speedup 1.0000x reference)
"""Distributed Trainium2 kernel for a decoder prompt layer (8 NeuronCores).

Sharding: hybrid batch x head tensor-parallel attention (each core: 1 batch,
4 heads = 2 head-pairs), token-parallel out-proj + FFN tail, joined by ONE
8-core AllToAll that routes each core exactly the query columns + head dims
it needs for its 512-token tail slice.

v2 restructure vs baseline:
  - single merged 1MB AllToAll (pays inter-core skew once, not twice)
  - x-input DMA prioritized ahead of weights; tail weights prefetched
    during attention so no DMA contends with the collective
  - QKV bias folded into the projection matmul (ones-row trick) and the
    PSUM->SBUF copy moved to DVE: scalar engine is reserved for softmax
    exp, which is the attention-phase bottleneck (~110us @ 1.2GHz)
  - pair-1 QKV projection chunks emitted as fillers INSIDE pair-0
    attention blocks so the in-order PE queue stays fed while ACT chews
    exps, without starving ACT between blocks
  - softmax-denominator broadcast moved from PE (rank-1 matmul) to the
    idle gpsimd engine
"""

import sys
from contextlib import ExitStack

sys.path.insert(0, "/opt/trn_rl_repo")

import numpy as np
import ml_dtypes

import concourse.bass as bass
import concourse.mybir as mybir
import concourse.tile as tile
from concourse import bacc
from concourse.masks import make_identity
from concourse import bass_utils

BF16 = ml_dtypes.bfloat16

B, T, M, D, H, DH, DF = 2, 2048, 256, 1024, 16, 64, 4096
R = DH // 2  # 32, rotary dims per head
S = M + T  # 2304
W = 8  # cores
EPS = 1e-5
BT = B * T  # 4096 flat decoder tokens
TPC = BT // W  # 512 tail tokens per core
NP = 2  # head pairs per core (4 heads, one batch per core)
NKB = S // 128  # 18 key blocks
NQB = T // 512  # 4 query blocks
DB = D // 128  # 8 contraction blocks of D
FB = DF // 128  # 32 blocks of DF

F32 = mybir.dt.float32
BF = mybir.dt.bfloat16
FP8 = mybir.dt.float8e4
E4 = ml_dtypes.float8_e4m3fn
ACTF = mybir.ActivationFunctionType
DR = mybir.MatmulPerfMode.DoubleRow

WSC = 64.0  # fp8 weight scale
OSC = 8.0  # fp8 attention-output scale

KERNEL_STATS = {}

_CACHED_NC = None


def _layernorm_tiles(nc, stat_pool, eps_t, y_t, xf_t, on_act=True):
    """LN over a (128, 1024) tile y_t -> normalized (no g/b) xf_t."""
    st = stat_pool.tile([128, 2, 6], F32, tag="bnst")
    nc.vector.bn_stats(out=st[:, 0, :], in_=y_t[:, 0:512])
    nc.vector.bn_stats(out=st[:, 1, :], in_=y_t[:, 512:1024])
    mv = stat_pool.tile([128, 2], F32, tag="mv")
    nc.vector.bn_aggr(out=mv[:], in_=st[:])
    rstd = stat_pool.tile([128, 1], F32, tag="rstd")
    nc.scalar.activation(
        out=rstd[:], in_=mv[:, 1:2], func=ACTF.Sqrt, bias=eps_t[:], scale=1.0
    )
    nc.vector.reciprocal(out=rstd[:], in_=rstd[:])
    nmr = stat_pool.tile([128, 1], F32, tag="nmr")
    nc.vector.tensor_mul(out=nmr[:], in0=mv[:, 0:1], in1=rstd[:])
    if on_act:
        nc.vector.tensor_scalar_mul(out=nmr[:], in0=nmr[:], scalar1=-1.0)
        nc.scalar.activation(
            out=xf_t[:], in_=y_t[:], func=ACTF.Identity, bias=nmr[:], scale=rstd[:]
        )
    else:
        nc.vector.tensor_scalar(
            out=xf_t[:],
            in0=y_t[:],
            scalar1=rstd[:],
            scalar2=nmr[:],
            op0=mybir.AluOpType.mult,
            op1=mybir.AluOpType.subtract,
        )


def _build_nc():
    nc = bacc.Bacc(trn_type="TRN2", debug=False, num_devices=W)

    io = {}
    io["xfull"] = nc.dram_tensor("xfull", [S, D], BF, kind="ExternalInput")
    for n in ("wq", "wk", "wv"):
        io[n] = nc.dram_tensor(n, [NP, 128, DB * 128], FP8, kind="ExternalInput")
    for n in ("bq", "bk", "bv"):
        # bias rows (pre-scaled by WSC), one [1,128] row per head-pair
        io[n] = nc.dram_tensor(n, [NP, 1, 128], BF, kind="ExternalInput")
    io["cos_k"] = nc.dram_tensor("cos_k", [128, S], BF, kind="ExternalInput")
    io["sin_k"] = nc.dram_tensor("sin_k", [128, S], BF, kind="ExternalInput")
    io["cos_q"] = nc.dram_tensor("cos_q", [128, T], BF, kind="ExternalInput")
    io["sin_q"] = nc.dram_tensor("sin_q", [128, T], BF, kind="ExternalInput")
    io["masks"] = nc.dram_tensor("masks", [4, 128, 1024], FP8, kind="ExternalInput")
    io["wo"] = nc.dram_tensor("wo", [128, DB * D], FP8, kind="ExternalInput")
    io["w1"] = nc.dram_tensor("w1", [FB, 128, DB * 128], BF, kind="ExternalInput")
    io["b1"] = nc.dram_tensor("b1", [128, FB], F32, kind="ExternalInput")
    io["w2"] = nc.dram_tensor("w2", [FB, 128, D], BF, kind="ExternalInput")
    io["b2"] = nc.dram_tensor("b2", [1, D], F32, kind="ExternalInput")
    io["xres"] = nc.dram_tensor("xres", [TPC, D], BF, kind="ExternalInput")
    io["out"] = nc.dram_tensor("out", [TPC, D], F32, kind="ExternalOutput")

    # single merged AllToAll buffer: 8 dest blocks x (2 pairs x 2 heads x 64)
    io["ata_in"] = nc.dram_tensor("ata_in", [W * 256, 512], FP8)
    io["ata_out"] = nc.dram_tensor("ata_out", [W * 256, 512], FP8)

    with tile.TileContext(nc) as tc:
        _emit(nc, tc, io)
    nc.compile()
    return nc


def _emit(nc, tc, io):
    xfull = io["xfull"].ap()
    out = io["out"].ap()
    xres = io["xres"].ap()

    # ---- persistent pools (left-stack enter order = reverse exit order) ----
    late_cm = tc.tile_pool(name="late", bufs=1)
    stat_cm = tc.tile_pool(name="stats", bufs=8)
    psum_cm = tc.tile_pool(name="psum", bufs=2, space="PSUM")
    psumw_cm = tc.tile_pool(name="psumw", bufs=2, space="PSUM")
    late2_cm = tc.tile_pool(name="late2", bufs=1)
    early_cm = tc.tile_pool(name="early", bufs=1)
    late = late_cm.__enter__()
    stat_pool = stat_cm.__enter__()
    psum = psum_cm.__enter__()
    psumw = psumw_cm.__enter__()
    late2 = late2_cm.__enter__()
    early = early_cm.__enter__()

    # attention-lifetime pools must sit below the short-lived ln pool on
    # the stacks (LIFO exit order)
    xaT_cm = tc.tile_pool(name="xaT", bufs=1, side="right")
    xaT_pool = xaT_cm.__enter__()
    rope_cm = tc.tile_pool(name="rope", bufs=4, side="right")
    rope_pool = rope_cm.__enter__()
    qkv_cm = tc.tile_pool(name="qkvT", bufs=1)
    qkv_pool = qkv_cm.__enter__()
    vtok_cm = tc.tile_pool(name="vtok", bufs=1)
    vtok_pool = vtok_cm.__enter__()

    # ---------- input x DMA first on sync (scalar/gpsimd queues kept
    # clear for ACT work and constant loads): LN1 is the start path ----
    ln_cm = tc.tile_pool(name="ln", bufs=6)
    ln_pool = ln_cm.__enter__()
    xin_view = xfull.rearrange("(c p) d -> p c d", p=128)
    x_tiles = []
    for ch in range(S // 128):
        x_t = ln_pool.tile([128, D], BF, tag="xin", name=f"xin{ch}")
        nc.sync.dma_start(out=x_t[:], in_=xin_view[:, ch, :])
        x_tiles.append(x_t)

    eps_t = late.tile([128, 1], F32, tag="eps")
    nc.vector.memset(eps_t, EPS)
    ident = late.tile([128, 128], BF, tag="ident")
    make_identity(nc, ident[:])
    onerow = late.tile([1, 512], BF, tag="onerow")
    nc.vector.memset(onerow, 1.0)

    def warm(n):
        # dummy weight loads: keep the PE activity monitor fed so the
        # HAM clock gate stays at 2.4GHz through sparse-PE stretches
        for _ in range(n):
            nc.tensor.ldweights(ident[:])

    # ---- attention constants: biases on scalar (tiny), bulk on gpsimd ----
    b_tiles = {}
    for n in ("bq", "bk", "bv"):
        b_tiles[n] = early.tile([1, NP, 128], BF, tag=n, name=n + "_t")
        nc.scalar.dma_start(
            out=b_tiles[n][:], in_=io[n].ap().rearrange("a p c -> p a c")
        )
    w_tiles = {}
    for n in ("wk", "wq", "wv"):
        w_tiles[n] = early.tile([128, NP, DB, 128], FP8, tag=n, name=n + "_t")
        nc.gpsimd.dma_start(
            out=w_tiles[n][:],
            in_=io[n].ap().rearrange("a p (d c) -> p a d c", c=128),
        )
    cosk_t = early.tile([128, S], BF, tag="cosk")
    sink_t = early.tile([128, S], BF, tag="sink")
    cosq_t = early.tile([128, T], BF, tag="cosq")
    sinq_t = early.tile([128, T], BF, tag="sinq")
    nc.gpsimd.dma_start(out=cosk_t[:], in_=io["cos_k"].ap())
    nc.gpsimd.dma_start(out=sink_t[:], in_=io["sin_k"].ap())
    nc.gpsimd.dma_start(out=cosq_t[:], in_=io["cos_q"].ap())
    nc.gpsimd.dma_start(out=sinq_t[:], in_=io["sin_q"].ap())
    mask_t = early.tile([128, 4, 1024], FP8, tag="masks")
    nc.gpsimd.dma_start(out=mask_t[:], in_=io["masks"].ap().rearrange("j p q -> p j q"))

    # ---- tail-weight prefetch on gpsimd queue (finishes mid-attention,
    # long before the A2A so nothing contends with the collective) ----
    b1_t = late2.tile([128, FB], F32, tag="b1")
    nc.gpsimd.dma_start(out=b1_t[:], in_=io["b1"].ap())
    b2row = late2.tile([1, D], F32, tag="b2row")
    nc.gpsimd.dma_start(out=b2row[:], in_=io["b2"].ap())
    b2_t = late2.tile([128, D], F32, tag="b2b")
    nc.gpsimd.partition_broadcast(out_ap=b2_t[:], in_ap=b2row[:])
    wo_t = late2.tile([128, DB, D], FP8, tag="wo")
    nc.gpsimd.dma_start(
        out=wo_t[:], in_=io["wo"].ap().rearrange("p (a c) -> p a c", c=D)
    )
    xr_t = late2.tile([128, 4, D], BF, tag="xr")
    nc.gpsimd.dma_start(out=xr_t[:], in_=xres.rearrange("(c p) d -> p c d", p=128))
    w2full = late2.tile([128, FB, D], BF, tag="w2full")
    nc.gpsimd.dma_start(out=w2full[:], in_=io["w2"].ap().rearrange("f p c -> p f c"))

    # ---------- tiles for LN output + QKV ----------
    xaT = xaT_pool.tile([128, DB, S], FP8, tag="xaT")
    qT = [qkv_pool.tile([128, T], BF, tag=f"qT{a}", name=f"qT{a}") for a in range(NP)]
    kT = [qkv_pool.tile([128, S], BF, tag=f"kT{a}", name=f"kT{a}") for a in range(NP)]
    vT = [qkv_pool.tile([128, S], BF, tag=f"vT{a}", name=f"vT{a}") for a in range(NP)]
    vtok = [
        vtok_pool.tile([128, NKB, 2, 80], FP8, tag=f"vtok{a}", name=f"vtok{a}")
        for a in range(NP)
    ]
    SWAP16 = list(range(16, 32)) + list(range(16))

    def proj_chunk(w_name, b_name, a, dst, src, dstc, on_act):
        """One projection chunk: dst[:, dstc] = (W.T x + b) / WSC."""
        cw = src[1] - src[0]
        ps = psum.tile([128, cw], F32, tag="ps", name="ps_proj")
        nc.tensor.matmul(
            ps[:], b_tiles[b_name][:, a, :], onerow[:, 0:cw], start=True, stop=False
        )
        for dbp in range(0, DB, 2):
            nc.tensor.matmul(
                ps[:],
                w_tiles[w_name][:, a, dbp : dbp + 2, :],
                xaT[:, dbp : dbp + 2, src[0] : src[1]],
                start=False,
                stop=(dbp == DB - 2),
                perf_mode=DR,
            )
        if on_act:
            nc.scalar.activation(
                out=dst[:, dstc[0] : dstc[1]],
                in_=ps[:],
                func=ACTF.Identity,
                scale=1.0 / WSC,
            )
        else:
            nc.vector.tensor_scalar_mul(
                out=dst[:, dstc[0] : dstc[1]], in0=ps[:], scalar1=1.0 / WSC
            )

    def rope_chunk(dst, cos_t, sin_t, c0, c1):
        cw = c1 - c0
        rot = rope_pool.tile([128, 512], BF, tag="rot", name="rot")
        t1 = rope_pool.tile([128, 512], BF, tag="t1", name="t1")
        nc.vector.stream_shuffle(out=rot[:, :cw], in_=dst[:, c0:c1], mask=SWAP16)
        nc.vector.tensor_mul(out=t1[:, :cw], in0=dst[:, c0:c1], in1=cos_t[:, c0:c1])
        nc.vector.tensor_mul(out=rot[:, :cw], in0=rot[:, :cw], in1=sin_t[:, c0:c1])
        nc.vector.tensor_add(out=dst[:, c0:c1], in0=t1[:, :cw], in1=rot[:, :cw])

    def vtok_chunk(a, kb):
        c0 = kb * 128
        for h in range(2):
            pt = psum.tile([128, 64], BF, tag="ps", name="ptr_v")
            nc.tensor.transpose(
                pt[:],
                vT[a][h * DH : (h + 1) * DH, c0 : c0 + 128],
                ident[h * DH : (h + 1) * DH, h * DH : (h + 1) * DH],
            )
            nc.vector.tensor_copy(out=vtok[a][:, kb, h, 0:64], in_=pt[:])

    k_chunks = []
    c0 = 0
    while c0 < S:
        c1 = min(c0 + 512, S)
        k_chunks.append(((c0, c1), (c0, c1)))
        c0 = c1
    q_chunks = [
        ((M + i * 512, M + (i + 1) * 512), (i * 512, (i + 1) * 512))
        for i in range(T // 512)
    ]

    def emit_pair_qkv(a, on_act):
        """K/Q/V proj + rope + vtok for one pair, yielding between chunks."""
        for src, dstc in k_chunks:
            proj_chunk("wk", "bk", a, kT[a], src, dstc, on_act)
            rope_chunk(kT[a], cosk_t, sink_t, dstc[0], dstc[1])
            yield
        for src, dstc in q_chunks:
            proj_chunk("wq", "bq", a, qT[a], src, dstc, on_act)
            rope_chunk(qT[a], cosq_t, sinq_t, dstc[0], dstc[1])
            yield
        nc.vector.memset(vtok[a][:, :, :, 64:65], 1.0 / OSC)
        for src, dstc in k_chunks:
            proj_chunk("wv", "bv", a, vT[a], src, dstc, on_act)
            yield
        for kb in range(NKB):
            vtok_chunk(a, kb)
            if kb % 3 == 2:
                yield

    # ---------- phase A: LN1 + transpose, with pair-0 K/Q projection
    # chunks pipelined in as their xaT column ranges complete ----------
    group_bounds = [(0, 512), (512, 1024), (1024, 1536), (1536, 2048), (2048, 2304)]
    for gi, (r0, r1) in enumerate(group_bounds):
        n = (r1 - r0) // 128
        xs = []
        for i in range(n):
            ch = r0 // 128 + i
            xa_t = ln_pool.tile([128, D], BF, tag="xaout")
            _layernorm_tiles(
                nc, stat_pool, eps_t, x_tiles[ch], xa_t, on_act=(ch % 2 == 0)
            )
            xs.append(xa_t)
        for db in range(DB):
            pt = psum.tile([128, 512], BF, tag="ps", name="ptr_ln")
            for i in range(n):
                nc.tensor.transpose(
                    pt[:, i * 128 : (i + 1) * 128],
                    xs[i][:, db * 128 : (db + 1) * 128],
                    ident[:],
                )
            # alternate the PSUM->SBUF cast between ACT and DVE to
            # balance the two engines in this DMA-paced phase
            if db % 2 == 0:
                nc.scalar.copy(out=xaT[:, db, r0:r1], in_=pt[:, 0 : r1 - r0])
            else:
                nc.vector.tensor_copy(out=xaT[:, db, r0:r1], in_=pt[:, 0 : r1 - r0])
        warm(10)
        if gi >= 1:
            src, dstc = k_chunks[gi - 1]
            proj_chunk("wk", "bk", 0, kT[0], src, dstc, True)
            rope_chunk(kT[0], cosk_t, sink_t, dstc[0], dstc[1])
            warm(4)
        if gi >= 2:
            src, dstc = q_chunks[gi - 2]
            proj_chunk("wq", "bq", 0, qT[0], src, dstc, True)
            rope_chunk(qT[0], cosq_t, sinq_t, dstc[0], dstc[1])
            warm(4)
    ln_cm.__exit__(None, None, None)

    # preload the exp ACT table set so the first attention exp doesn't
    # eat the ~2.7us table switch
    dummy = late.tile([128, 1], F32, tag="dummy")
    nc.scalar.activation(out=dummy[:], in_=eps_t[:], func=ACTF.Exp)

    # finish pair 0: last K chunk, last 2 Q chunks, V, vtok
    src, dstc = k_chunks[4]
    proj_chunk("wk", "bk", 0, kT[0], src, dstc, True)
    rope_chunk(kT[0], cosk_t, sink_t, dstc[0], dstc[1])
    warm(4)
    for qi in (2, 3):
        src, dstc = q_chunks[qi]
        proj_chunk("wq", "bq", 0, qT[0], src, dstc, True)
        rope_chunk(qT[0], cosq_t, sinq_t, dstc[0], dstc[1])
        warm(4)
    nc.vector.memset(vtok[0][:, :, :, 64:65], 1.0 / OSC)
    for src, dstc in k_chunks:
        proj_chunk("wv", "bv", 0, vT[0], src, dstc, True)
        warm(3)
    for kb in range(NKB):
        vtok_chunk(0, kb)

    # ---------- attention ----------
    attn_cm = tc.tile_pool(name="attnT", bufs=4)
    attn_pool = attn_cm.__enter__()
    nrm_cm = tc.tile_pool(name="nrm", bufs=3)
    nrm_pool = nrm_cm.__enter__()
    pid_sy = nc.sync.partition_id()
    grp4_sy = pid_sy & 4
    ata_in = io["ata_in"].ap()

    # init the 4 rotating aa slots so narrowed-exp's untouched columns
    # hold finite fp8 (mask-mul zeroes them; garbage NaN would poison PV)
    for _ in range(4):
        aainit = attn_pool.tile([128, 2, 1024], FP8, tag="at", name="aainit")
        nc.vector.memset(aainit[:], 0.0)

    def attn_block(a, qb, filler=None):
        nk = M // 128 + 4 * (qb + 1)
        qc0 = qb * 512
        po = [psum.tile([65, 512], F32, tag="po", name="po") for _ in range(2)]
        aa = None
        for kb in range(nk):
            kc0 = kb * 128
            pss = psumw.tile([128, 1024], F32, tag="psw", name="pss")
            for h in range(2):
                h0 = h * DH
                nc.tensor.matmul(
                    pss[:, h * 512 : (h + 1) * 512],
                    kT[a][h0 : h0 + DH, kc0 : kc0 + 128],
                    qT[a][h0 : h0 + DH, qc0 : qc0 + 512],
                    start=True,
                    stop=True,
                )
            if kb % 2 == 0:
                aa = attn_pool.tile([128, 2, 1024], FP8, tag="at")
            jm = kb - (nk - 4)
            if jm <= 0:
                nc.scalar.activation(out=aa[:, kb % 2, :], in_=pss[:], func=ACTF.Exp)
            else:
                # diagonal block: columns < 128*jm are fully masked; skip
                # their exp (mask-mul zeroes whatever is there)
                off = 128 * jm
                nc.scalar.activation(
                    out=aa[:, kb % 2, :].rearrange("p (h q) -> p h q", h=2)[
                        :, :, off:
                    ],
                    in_=pss[:].rearrange("p (h q) -> p h q", h=2)[:, :, off:],
                    func=ACTF.Exp,
                )
            if jm >= 0:
                nc.vector.tensor_mul(
                    out=aa[:, kb % 2, :], in0=aa[:, kb % 2, :], in1=mask_t[:, jm, :]
                )
            if kb % 2 == 1:
                # fp8 DoubleRow PV over the key-block pair
                for h in range(2):
                    nc.tensor.matmul(
                        po[h][:],
                        vtok[a][:, kb - 1 : kb + 1, h, 0:65],
                        aa[:, :, h * 512 : (h + 1) * 512],
                        start=(kb == 1),
                        stop=(kb == nk - 1),
                        perf_mode=DR,
                    )
                # keep the in-order PE queue fed while ACT chews exps
                if filler is not None:
                    try:
                        next(filler)
                    except StopIteration:
                        filler = None
                if filler is None:
                    warm(8)
        # softmax normalize: denom row -> SBUF -> gpsimd broadcast ->
        # fast reciprocal -> multiply; ship to the A2A staging buffer
        for h in range(2):
            zsb = nrm_pool.tile([1, 512], BF, tag="zsb")
            nc.vector.tensor_copy(out=zsb[:], in_=po[h][64:65, :])
            # PE rank-1 broadcast keeps the gpsimd queue empty so the
            # A2A trigger posts with minimal delay (and adds MAC activity)
            pb = psumw.tile([64, 512], F32, tag="psw", name="pb")
            nc.tensor.matmul(pb[:], onerow[:, 0:64], zsb[:], start=True, stop=True)
            recb = nrm_pool.tile([64, 512], F32, tag="recb")
            nc.vector.tensor_copy(out=recb[:], in_=pb[:])
            nc.vector.reciprocal_approx_fast(out=recb[:], in_=recb[:])
            onorm = nrm_pool.tile([64, 512], FP8, tag="onorm")
            nc.vector.tensor_mul(out=onorm[:, :], in0=po[h][0:64, :], in1=recb[:, :])
            nc.sync.dma_start(
                out=ata_in[
                    bass.ds((grp4_sy + qb) * 256 + a * 128 + h * DH, DH), :
                ],
                in_=onorm[:, :],
            )
        return filler

    # pair-1 QKV rides inside pair-0 attention blocks
    p1_steps = emit_pair_qkv(1, on_act=False)
    for qb in range(NQB):
        p1_steps = attn_block(0, qb, p1_steps)
    if p1_steps is not None:
        for _ in p1_steps:
            pass
    for qb in range(NQB):
        attn_block(1, qb)

    # ---------- single merged AllToAll (both pairs, all qbs) ----------
    nc.gpsimd.collective_compute(
        "AllToAll",
        mybir.AluOpType.bypass,
        replica_groups=[list(range(W))],
        ins=[io["ata_in"].ap()],
        outs=[io["ata_out"].ap()],
    )
    # keep the PE warm through the collective so the tail starts at full
    # clock (dummy weight loads have no deps and run during the A2A)
    warm(120)

    nrm_cm.__exit__(None, None, None)
    attn_cm.__exit__(None, None, None)
    vtok_cm.__exit__(None, None, None)
    qkv_cm.__exit__(None, None, None)
    rope_cm.__exit__(None, None, None)  # right stack top
    xaT_cm.__exit__(None, None, None)  # right stack
    early_cm.__exit__(None, None, None)

    # ---------- gather ofT: my 4 sources' 16 head-blocks for my tokens ----
    # (gpsimd queue: in-order after the collective completes)
    pid_gp = nc.gpsimd.partition_id()
    grp4_gp = pid_gp & 4
    ofT_cm = tc.tile_pool(name="ofT", bufs=1, side="right")
    ofT_pool = ofT_cm.__enter__()
    ofT = ofT_pool.tile([128, DB, TPC], FP8, tag="ofT")
    agv = io["ata_out"].ap().rearrange("(s r p) q -> p (s r) q", r=2, p=128)
    nc.gpsimd.dma_start(out=ofT[:], in_=agv[:, bass.ds(grp4_gp * 2, 8), :])

    # ---------- out-proj + residual + LN2 + xfT transpose ----------
    y_cm = tc.tile_pool(name="y", bufs=1)
    y_pool = y_cm.__enter__()
    xfT_cm = tc.tile_pool(name="xfT", bufs=1)
    xfT_pool = xfT_cm.__enter__()
    xfT = xfT_pool.tile([128, DB, TPC], BF, tag="xfT")
    ln2_cm = tc.tile_pool(name="ln2", bufs=4)
    ln2_pool = ln2_cm.__enter__()
    y_tiles = []
    for tt in range(TPC // 128):
        pz = [psum.tile([128, 512], F32, tag="ps", name="pz") for _ in range(2)]
        for dbp in range(0, DB, 2):
            for half in range(2):
                nc.tensor.matmul(
                    pz[half][:],
                    ofT[:, dbp : dbp + 2, tt * 128 : (tt + 1) * 128],
                    wo_t[:, dbp : dbp + 2, half * 512 : (half + 1) * 512],
                    start=(dbp == 0),
                    stop=(dbp == DB - 2),
                    perf_mode=DR,
                )
        y_t = y_pool.tile([128, D], F32, tag=f"y{tt}", name=f"y{tt}")
        for half in range(2):
            hs = slice(half * 512, (half + 1) * 512)
            nc.vector.scalar_tensor_tensor(
                out=y_t[:, hs],
                in0=pz[half][:],
                scalar=1.0 / (WSC * OSC),
                in1=xr_t[:, tt, hs],
                op0=mybir.AluOpType.mult,
                op1=mybir.AluOpType.add,
            )
        y_tiles.append(y_t)
        warm(8)
        xf_t = ln2_pool.tile([128, D], BF, tag="xf")
        _layernorm_tiles(nc, stat_pool, eps_t, y_t, xf_t)
        pxf = psum.tile([128, DB, 128], BF, tag="ps", name="pxf")
        for db in range(DB):
            nc.tensor.transpose(
                pxf[:, db, :],
                xf_t[:, db * 128 : (db + 1) * 128],
                ident[:],
            )
        nc.vector.tensor_copy(out=xfT[:, :, tt * 128 : (tt + 1) * 128], in_=pxf[:])
    ln2_cm.__exit__(None, None, None)

    # ---------- FFN1 (h = relu(xf@W1+b1)^2, DF-major) ----------
    h2_cm = tc.tile_pool(name="h2T", bufs=1)
    h2_pool = h2_cm.__enter__()
    w1_cm = tc.tile_pool(name="w1c", bufs=2)
    w1_pool = w1_cm.__enter__()
    hr_cm = tc.tile_pool(name="hr", bufs=4)
    hr_pool = hr_cm.__enter__()
    h2T = h2_pool.tile([128, FB, TPC], BF, tag="h2T")
    w1_ap = io["w1"].ap()
    for fbg in range(FB // 8):
        w1c = w1_pool.tile([128, 8, DB, 128], BF, tag="w1c")
        nc.sync.dma_start(
            out=w1c[:],
            in_=w1_ap.rearrange("(g f) p (a c) -> p g f a c", f=8, c=128)[
                :, fbg, :, :, :
            ],
        )
        for fi in range(8):
            fb = fbg * 8 + fi
            ph = psum.tile([128, TPC], F32, tag="ps", name="ph")
            for db in range(DB):
                nc.tensor.matmul(
                    ph[:],
                    w1c[:, fi, db, :],
                    xfT[:, db, :],
                    start=(db == 0),
                    stop=(db == DB - 1),
                )
            hr = hr_pool.tile([128, TPC], BF, tag="hr")
            nc.scalar.activation(
                out=hr[:],
                in_=ph[:],
                func=ACTF.Relu,
                bias=b1_t[:, fb : fb + 1],
                scale=1.0,
            )
            nc.vector.tensor_mul(out=h2T[:, fb, :], in0=hr[:], in1=hr[:])
    hr_cm.__exit__(None, None, None)
    w1_cm.__exit__(None, None, None)

    # ---------- FFN2 + residual + store ----------
    out_cm = tc.tile_pool(name="outp", bufs=2)
    out_pool = out_cm.__enter__()
    for tt in range(TPC // 128):
        pz = [psum.tile([128, 512], F32, tag="ps", name="pz2") for _ in range(2)]
        for fb in range(FB):
            for half in range(2):
                nc.tensor.matmul(
                    pz[half][:],
                    h2T[:, fb, tt * 128 : (tt + 1) * 128],
                    w2full[:, fb, half * 512 : (half + 1) * 512],
                    start=(fb == 0),
                    stop=(fb == FB - 1),
                )
        o_t = out_pool.tile([128, D], F32, tag="ot")
        for half in range(2):
            hs = slice(half * 512, (half + 1) * 512)
            nc.vector.tensor_add(
                out=o_t[:, hs], in0=pz[half][:], in1=y_tiles[tt][:, hs]
            )
            nc.vector.tensor_add(out=o_t[:, hs], in0=o_t[:, hs], in1=b2_t[:, hs])
        nc.sync.dma_start(out=out[tt * 128 : (tt + 1) * 128, :], in_=o_t[:])
    out_cm.__exit__(None, None, None)
    h2_cm.__exit__(None, None, None)
    xfT_cm.__exit__(None, None, None)
    y_cm.__exit__(None, None, None)
    ofT_cm.__exit__(None, None, None)  # right stack
    late2_cm.__exit__(None, None, None)
    stat_cm.__exit__(None, None, None)
    late_cm.__exit__(None, None, None)
    psumw_cm.__exit__(None, None, None)
    psum_cm.__exit__(None, None, None)


def _prep_inputs(x, memory, Wq, bq, Wk, bk, Wv, bv, Wo, bo, W1, b1, W2, b2,
                 ln1_g, ln1_b, ln2_g, ln2_b):
    f32 = np.float32
    x = np.asarray(x, f32)
    memory = np.asarray(memory, f32)
    x_full_b = np.concatenate([memory, x], axis=1)  # (B, S, D)

    g1 = np.asarray(ln1_g, f32)
    b1n = np.asarray(ln1_b, f32)
    g2 = np.asarray(ln2_g, f32)
    b2n = np.asarray(ln2_b, f32)

    # split the 1/sqrt(DH) score scale evenly between q and k so both
    # sit in a healthy fp8 range
    scale_q = np.float32(np.float32(DH) ** -0.25)
    Wq_e = (g1[:, None] * np.asarray(Wq, f32)) * scale_q
    bq_e = (b1n @ np.asarray(Wq, f32) + np.asarray(bq, f32)) * scale_q
    Wk_e = (g1[:, None] * np.asarray(Wk, f32)) * scale_q
    bk_e = (b1n @ np.asarray(Wk, f32) + np.asarray(bk, f32)) * scale_q
    Wv_e = g1[:, None] * np.asarray(Wv, f32)
    bv_e = b1n @ np.asarray(Wv, f32) + np.asarray(bv, f32)
    W1_e = g2[:, None] * np.asarray(W1, f32)
    b1_e = b2n @ np.asarray(W1, f32) + np.asarray(b1, f32)

    inv_freq = 1.0 / (10000.0 ** (np.arange(0, R, 2, dtype=f32) / np.float32(R)))
    t = np.arange(S, dtype=f32)
    freqs = t[:, None] * inv_freq[None, :]
    emb = np.concatenate([freqs, freqs], axis=-1)  # (S, R)
    cos = np.cos(emb).T.astype(f32)  # (R, S)
    sin = np.sin(emb).T.astype(f32)
    ssin = np.concatenate([-sin[:16], sin[16:]], axis=0)

    def wide(tab, fill):
        o = np.full((128, tab.shape[1]), fill, np.float32)
        o[0:R] = tab
        o[64 : 64 + R] = tab
        return o

    cos_k = wide(cos, 1.0).astype(BF16)
    sin_k = wide(ssin, 0.0).astype(BF16)
    cos_q = wide(cos[:, M:], 1.0).astype(BF16)
    sin_q = wide(ssin[:, M:], 0.0).astype(BF16)

    kk = np.arange(128)[:, None]
    qq = np.arange(512)[None, :]
    mask = np.stack([(qq >= 128 * j + kk) for j in range(4)]).astype(E4)
    mask = np.concatenate([mask, mask], axis=2)  # (4, 128, 1024) both heads

    wo_arr = np.asarray(Wo, f32) * WSC
    wo_host = (
        np.ascontiguousarray(wo_arr.reshape(DB, 128, D).transpose(1, 0, 2))
        .reshape(128, DB * D)
        .astype(E4)
    )
    w1_host = (
        np.ascontiguousarray(W1_e.reshape(DB, 128, FB, 128).transpose(2, 1, 0, 3))
        .reshape(FB, 128, DB * 128)
        .astype(BF16)
    )
    w2_host = np.asarray(W2, f32).reshape(FB, 128, D).astype(BF16)
    b1_host = np.ascontiguousarray(b1_e.reshape(FB, 128).T).astype(f32)
    b2_host = np.ascontiguousarray(np.asarray(b2, f32).reshape(1, D))

    bo_arr = np.asarray(bo, f32)
    x_dec = x.reshape(BT, D)

    in_maps = []
    for c in range(W):
        g = c // 4  # batch handled by this core
        p = c % 4  # group position -> global heads 4p..4p+4
        hcols = slice(p * 256, (p + 1) * 256)

        def wslice(We):
            # (1024, 256) -> (NP, 128, DB*128), head-pair major
            wc = We[:, hcols].reshape(DB, 128, NP, 128) * WSC
            return (
                np.ascontiguousarray(wc.transpose(2, 1, 0, 3))
                .reshape(NP, 128, DB * 128)
                .astype(E4)
            )

        def bslice(be):
            # bias rows, pre-scaled by WSC to match fp8-scaled psum
            return np.ascontiguousarray(
                (be[hcols] * WSC).reshape(NP, 1, 128)
            ).astype(BF16)

        xres_c = x_dec[c * TPC : (c + 1) * TPC] + bo_arr[None, :]
        in_maps.append(
            {
                "xfull": x_full_b[g].astype(BF16),
                "wq": wslice(Wq_e),
                "wk": wslice(Wk_e),
                "wv": wslice(Wv_e),
                "bq": bslice(bq_e),
                "bk": bslice(bk_e),
                "bv": bslice(bv_e),
                "cos_k": cos_k,
                "sin_k": sin_k,
                "cos_q": cos_q,
                "sin_q": sin_q,
                "masks": mask,
                "wo": wo_host,
                "w1": w1_host,
                "b1": b1_host,
                "w2": w2_host,
                "b2": b2_host,
                "xres": np.ascontiguousarray(xres_c).astype(BF16),
            }
        )
    return in_maps


def kernel(**inputs):
    global _CACHED_NC
    if _CACHED_NC is None:
        _CACHED_NC = _build_nc()
    nc = _CACHED_NC
    in_maps = _prep_inputs(**inputs)
    res = bass_utils.run_bass_kernel_spmd(nc, in_maps, core_ids=list(range(W)))
    KERNEL_STATS["exec_time_ns"] = res.exec_time_ns
    KERNEL_STATS["mean_exec_time_ns"] = res.mean_exec_time_ns
    KERNEL_STATS["trace_path"] = (
        res.instructions_and_trace[1] if res.instructions_and_trace else None
    )
    KERNEL_STATS["profile_json"] = res.profile_json
    outs = np.stack([res.results[c]["out"] for c in range(W)])  # (W, TPC, D)
    return outs.reshape(B, T, D).astype(np.float32)
